# revision 1
# baseline (speedup 1.0000x reference)
"""Trainium2 Bass kernel for nn_HallucinationDetector.

Computes, per batch sample b:
    risk[b] = clip( 0.25 * routing_entropy[b]/ln(8)
                  - 0.2  * (1 - moe_confidence[b])
                  + 0.2  * sigmoid(memory_mismatch[b] - 2)
                  + 0.2  * mean_s sigmoid(hidden[b,s,:] @ probe_w + probe_b)
                  + 0.15 * sigmoid(1/(||routing_repr[b]|| + 1e-8) - 1), 0, 1)

Sharding: pure data-parallel over the batch dim across 8 NeuronCores
(128 samples per core). Layout on each core: partition = local sample
index (exactly 128), free dim = (seq, hidden).

The dominant cost is streaming 128 MiB/core of hidden_states from HBM.
Measured on this hardware the stream runs at ~420-450 GB/s effective
when each DMA's source is fully contiguous, so the host pre-relayouts
hidden_states tile-major ([n_tiles, BPC, s_tile*H]) and the kernel
issues many small (1 MiB) deeply-buffered DMAs. The per-token probe
dot products are fused DVE ops (multiply + free-dim reduce in one
instruction, ~537 ns/token) and hide completely under the DMA stream.
The sigmoid+mean over sequence is chunked so it overlaps the stream,
leaving only the last chunk + the tiny fusion chain on the tail.
"""

import math
from contextlib import ExitStack

import numpy as np

import concourse.bass as bass
import concourse.bacc as bacc
import concourse.tile as tile
from concourse import mybir
from concourse.bass_utils import run_bass_kernel_spmd

# Problem shapes (hardcoded; kernel.py must be self-contained).
B, S, H, D = 1024, 512, 512, 2048
N_CORES = 8
BPC = B // N_CORES  # 128 samples per core == SBUF partition count

MAX_ENTROPY = math.log(8.0)
W_ENTROPY, W_CONF, W_MISMATCH, W_SEMANTIC, W_EIGEN = 0.25, -0.2, 0.2, 0.2, 0.15

F32 = mybir.dt.float32
BF16 = mybir.dt.bfloat16

DEFAULTS = dict(s_tile=8, hid_bufs=12, contig=True, use_bf16=True)


def build_nc(
    s_tile: int = DEFAULTS["s_tile"],
    use_bf16: bool = DEFAULTS["use_bf16"],
    hid_bufs: int = DEFAULTS["hid_bufs"],
    repeats: int = 1,
    no_compute: bool = False,
    no_dma: bool = False,
    alt_rings: bool = False,
    contig: bool = DEFAULTS["contig"],
    sem_chunks: int = 4,
):
    """Build the per-core Bass program. Identical on every core (pure SPMD,
    no collectives); each core sees its own 128-sample shard.

    repeats > 1 re-runs the whole body N times (same result) — a timing
    device: HW time/iter = (wall(R2) - wall(R1)) / (R2 - R1)."""
    nc = bacc.Bacc("TRN2", target_bir_lowering=False, debug=True)

    n_tiles = S // s_tile
    assert S % s_tile == 0 and S % sem_chunks == 0
    chunk_cols = S // sem_chunks
    assert chunk_cols % s_tile == 0
    tiles_per_chunk = n_tiles // sem_chunks

    if contig:
        # tile-major host relayout: hid[t, b, s_in_tile*H] with each tile's
        # slab fully contiguous in DRAM (one linear read per DMA).
        hid = nc.dram_tensor(
            "hidden_states", [n_tiles, BPC, s_tile * H], F32, kind="ExternalInput"
        )
    else:
        hid = nc.dram_tensor("hidden_states", [BPC, S, H], F32, kind="ExternalInput")
    rr = nc.dram_tensor("routing_repr", [BPC, D], F32, kind="ExternalInput")
    re = nc.dram_tensor("routing_entropy", [BPC], F32, kind="ExternalInput")
    mc = nc.dram_tensor("moe_confidence", [BPC], F32, kind="ExternalInput")
    mm = nc.dram_tensor("memory_mismatch", [BPC], F32, kind="ExternalInput")
    pw = nc.dram_tensor("probe_w", [H], F32, kind="ExternalInput")
    pb = nc.dram_tensor("probe_b", [1], F32, kind="ExternalInput")
    risk = nc.dram_tensor("risk", [BPC], F32, kind="ExternalOutput")

    dt_main = BF16 if use_bf16 else F32
    mult, add = mybir.AluOpType.mult, mybir.AluOpType.add

    with ExitStack() as ctx:
        tc = ctx.enter_context(tile.TileContext(nc))
        singles = ctx.enter_context(tc.tile_pool(name="singles", bufs=1))
        hid_pool = ctx.enter_context(tc.tile_pool(name="hid", bufs=hid_bufs))

      # fmt: off
        def body():
            # Small loads go on whichever DGE path the hid stream does NOT
            # use: SWDGE (gpsimd) for the f32/HWDGE stream, HWDGE (sync) for
            # the bf16/SWDGE-cast stream. The w broadcast needs SWDGE (step-0
            # partition AP + optional cast) in both cases.
            small_eng = nc.sync if use_bf16 else nc.gpsimd

            # probe_w broadcast to all 128 partitions via a step-0 partition AP.
            w_tile = singles.tile([BPC, H], dt_main)
            w_src = pw[:]
            w_bcast = bass.AP(
                tensor=w_src.tensor, offset=w_src.offset, ap=[[0, BPC]] + list(w_src.ap)
            )
            nc.gpsimd.dma_start(out=w_tile, in_=w_bcast)

            b_tile = singles.tile([BPC, 1], F32)
            b_src = pb[:]
            b_bcast = bass.AP(
                tensor=b_src.tensor, offset=b_src.offset, ap=[[0, BPC]] + list(b_src.ap)
            )
            nc.gpsimd.dma_start(out=b_tile, in_=b_bcast)

            # --- small per-sample vectors: [128] dram -> [128, 1] sbuf ---------
            def load_col(src, nm):
                t = singles.tile([BPC, 1], F32, name=nm, tag=nm)
                small_eng.dma_start(out=t, in_=src[:].rearrange("(p o) -> p o", o=1))
                return t

            re_t = load_col(re, "re_t")
            mc_t = load_col(mc, "mc_t")
            mm_t = load_col(mm, "mm_t")

            # --- eigen-score branch (overlaps the hid stream) ------------------
            rr_t = singles.tile([BPC, D], F32)
            small_eng.dma_start(out=rr_t, in_=rr[:, :])
            rr_scr = singles.tile([BPC, D], F32)
            ss = singles.tile([BPC, 1], F32)
            # fused square + free-dim-sum: out = (rr*1.0)*rr, accum_out = sum(out)
            nc.vector.scalar_tensor_tensor(
                out=rr_scr, in0=rr_t, scalar=1.0, in1=rr_t,
                op0=mult, op1=mult, accum_out=ss,
            )
            sv = singles.tile([BPC, 1], F32)
            nc.scalar.sqrt(sv, ss)
            nc.vector.tensor_scalar_add(sv, sv, 1e-8)
            eig = singles.tile([BPC, 1], F32)
            nc.vector.reciprocal(eig, sv)
            neg1 = singles.tile([BPC, 1], F32)
            nc.vector.memset(neg1, -1.0)
            neg2 = singles.tile([BPC, 1], F32)
            nc.vector.memset(neg2, -2.0)
            ne2 = singles.tile([BPC, 1], F32)
            nc.scalar.activation(
                ne2, eig, mybir.ActivationFunctionType.Sigmoid, bias=neg1, scale=1.0
            )
            nm_t = singles.tile([BPC, 1], F32)
            nc.scalar.activation(
                nm_t, mm_t, mybir.ActivationFunctionType.Sigmoid, bias=neg2, scale=1.0
            )

            # --- partial fusion (everything except the semantic term) ----------
            # a_pre = (W_E/ln8)*re + W_CONF + (-W_CONF)*mc + W_M*nm + W_EIG*ne2
            _fma_n = [0]

            def fma(x, c, acc):
                _fma_n[0] += 1
                o = singles.tile([BPC, 1], F32, name=f"fma{_fma_n[0]}",
                                 tag=f"fma{_fma_n[0]}")
                nc.vector.scalar_tensor_tensor(
                    out=o, in0=x, scalar=c, in1=acc, op0=mult, op1=add
                )
                return o

            a0 = singles.tile([BPC, 1], F32)
            nc.vector.tensor_scalar(
                out=a0, in0=re_t, scalar1=W_ENTROPY / MAX_ENTROPY, scalar2=W_CONF,
                op0=mult, op1=add,
            )
            a1 = fma(mc_t, -W_CONF, a0)
            a2 = fma(nm_t, W_MISMATCH, a1)
            a_pre = fma(ne2, W_EIGEN, a2)

            # --- main loop: logits[b, s] = hidden[b, s, :] @ probe_w -----------
            # logits split into sem_chunks disjoint tiles so each chunk's
            # sigmoid+reduce can run while later tiles still stream in.
            logit_chunks = [
                singles.tile([BPC, chunk_cols], F32, name=f"logits{k}",
                             tag=f"logits{k}")
                for k in range(sem_chunks)
            ]
            sum_chunks = []
            scr = singles.tile([BPC, H], dt_main)  # TTR's (unused) full output
            dma_eng = nc.gpsimd if use_bf16 else nc.sync  # cast needs SWDGE
            if no_compute:
                for lt in logit_chunks:
                    nc.vector.memset(lt, 0.0)
            static_ht = None
            if no_dma:
                static_ht = hid_pool.tile([BPC, s_tile, H], dt_main, tag="static_ht")
                nc.vector.memset(static_ht, 0.01)

            def emit_chunk_sem(k):
                pk = singles.tile([BPC, chunk_cols], F32, name=f"probs{k}",
                                  tag=f"probs{k}")
                nc.scalar.activation(
                    pk, logit_chunks[k], mybir.ActivationFunctionType.Sigmoid,
                    bias=b_tile, scale=1.0,
                )
                sk = singles.tile([BPC, 1], F32, name=f"sum{k}", tag=f"sum{k}")
                nc.vector.tensor_reduce(sk, pk, mybir.AxisListType.X, add)
                sum_chunks.append(sk)

            for i in range(n_tiles):
                if no_dma:
                    ht = static_ht
                else:
                    ht = hid_pool.tile([BPC, s_tile, H], dt_main)
                    eng = dma_eng
                    if alt_rings and not use_bf16:
                        eng = nc.sync if (i % 2 == 0) else nc.scalar
                    if contig:
                        eng.dma_start(
                            out=ht.rearrange("p a b -> p (a b)"), in_=hid[i, :, :]
                        )
                    else:
                        eng.dma_start(
                            out=ht, in_=hid[:, i * s_tile : (i + 1) * s_tile, :]
                        )
                if not no_compute:
                    for j in range(s_tile):
                        s_idx = i * s_tile + j
                        k, col = divmod(s_idx, chunk_cols)
                        # fused dot product: out = (h*1.0)*w, accum = sum(out)
                        nc.vector.scalar_tensor_tensor(
                            out=scr, in0=ht[:, j, :], scalar=1.0, in1=w_tile,
                            op0=mult, op1=mult,
                            accum_out=logit_chunks[k][:, col : col + 1],
                        )
                if (i + 1) % tiles_per_chunk == 0:
                    emit_chunk_sem((i + 1) // tiles_per_chunk - 1)

            # --- combine chunk sums + final fusion + clip ----------------------
            while len(sum_chunks) > 1:
                nxt = []
                for a in range(0, len(sum_chunks) - 1, 2):
                    _fma_n[0] += 1
                    o = singles.tile([BPC, 1], F32, name=f"csum{_fma_n[0]}",
                                     tag=f"csum{_fma_n[0]}")
                    nc.vector.tensor_tensor(
                        out=o, in0=sum_chunks[a], in1=sum_chunks[a + 1], op=add
                    )
                    nxt.append(o)
                if len(sum_chunks) % 2:
                    nxt.append(sum_chunks[-1])
                sum_chunks[:] = nxt
            sum_p = sum_chunks[0]

            a_fin = fma(sum_p, W_SEMANTIC / S, a_pre)
            out_t = singles.tile([BPC, 1], F32)
            nc.vector.tensor_scalar(
                out=out_t, in0=a_fin, scalar1=0.0, scalar2=1.0,
                op0=mybir.AluOpType.max, op1=mybir.AluOpType.min,
            )
            nc.sync.dma_start(out=risk[:].rearrange("(p o) -> p o", o=1), in_=out_t)

        for _rep in range(repeats):
            body()

    nc.finalize()
    return nc


_NC_CACHE: dict = {}


def _get_nc(**kw):
    key = tuple(sorted(kw.items()))
    if key not in _NC_CACHE:
        _NC_CACHE[key] = build_nc(**kw)
    return _NC_CACHE[key]


def _make_in_maps(
    inputs: dict,
    contig: bool = DEFAULTS["contig"],
    s_tile: int = DEFAULTS["s_tile"],
) -> list:
    hs = np.ascontiguousarray(np.asarray(inputs["hidden_states"], dtype=np.float32))
    rr = np.ascontiguousarray(np.asarray(inputs["routing_repr"], dtype=np.float32))
    re = np.asarray(inputs["routing_entropy"], dtype=np.float32)
    mc = np.asarray(inputs["moe_confidence"], dtype=np.float32)
    mm = np.asarray(inputs["memory_mismatch"], dtype=np.float32)
    pw = np.asarray(inputs["probe_w"], dtype=np.float32)
    pb = np.asarray(inputs["probe_b"], dtype=np.float32)
    maps = []
    for c in range(N_CORES):
        sl = slice(c * BPC, (c + 1) * BPC)
        hs_c = hs[sl]
        if contig:
            # [BPC, S, H] -> [S//s_tile, BPC, s_tile*H], tile-major contiguous
            hs_c = np.ascontiguousarray(
                hs_c.reshape(BPC, S // s_tile, s_tile * H).transpose(1, 0, 2)
            )
        maps.append(
            {
                "hidden_states": hs_c,
                "routing_repr": rr[sl],
                "routing_entropy": re[sl],
                "moe_confidence": mc[sl],
                "memory_mismatch": mm[sl],
                "probe_w": pw,
                "probe_b": pb,
            }
        )
    return maps


def run(inputs: dict, trace: bool = False, **build_kw):
    """Run the kernel on 8 cores; returns (risk[1024] f32, BassKernelResults)."""
    nc = _get_nc(**build_kw)
    in_maps = _make_in_maps(
        inputs,
        contig=build_kw.get("contig", DEFAULTS["contig"]),
        s_tile=build_kw.get("s_tile", DEFAULTS["s_tile"]),
    )
    res = run_bass_kernel_spmd(nc, in_maps, list(range(N_CORES)), trace=trace)
    out = np.concatenate([res.results[c]["risk"] for c in range(N_CORES)])
    return out.astype(np.float32, copy=False), res


def kernel(**inputs) -> np.ndarray:
    out, _ = run(inputs)
    return out

